# revision 35
# baseline (speedup 1.0000x reference)
"""Multi-head self-attention (B=4, S=1024, D=1024, H=16, RoPE, causal) on 8
Trainium2 NeuronCores.

Sharding: 8 cores = 4 batches x 2 head-groups (8 heads each). Each core
computes QKV projections for its batch/head-group, RoPE, causal attention,
and a partial output projection (contraction over its 512 attention dims).
The host sums the two partial outputs per batch (the "all-reduce") and
concatenates batches.

Design (v4):
- All matmul operands bf16 (1 cyc/row at any tile size; half the DMA).
- Q/K are projected TRANSPOSED (weights stationary, x streamed): no PE
  transposes. RoPE runs in [dg, s] layout; the even/odd partner rows come
  from a PE permutation matmul (psw), then 3 DVE elementwise ops.
- Causal attention splits into two q-halves. Half-0 (q<512, k-tiles 0..3)
  interleaves with the second half of the projections so ACT exp time hides
  under PE projection time. The transposed-logits layout (L^T[k,q]) gives
  softmax sums via a ones column in V.
- Softmax 1/sum per head pair: two s-rows batched through one reshape DMA,
  one [128,8] reciprocal, one broadcast DMA; normalization multiplies write
  even heads to partitions 0:64 and odd heads directly to 64:128
  (cross-partition-offset engine writes work at 32-partition granularity).
- tril masking on GpSimd (idle otherwise); exp on ACT; everything else
  balanced between DVE/ACT per phase.
- Output projection per q-half interleaves with the other half's attention;
  y is written bf16 (host sums the two partial outputs in f32).
"""

import numpy as np

import concourse.bass as bass
import concourse.mybir as mybir
import concourse.tile as tile
from concourse.bass import ts
from concourse.bass_utils import run_bass_kernel_spmd
from concourse.masks import make_upper_triangular

B, S, D = 4, 1024, 1024
H = 16  # total heads
HG = 8  # heads per core (head-group)
DK = 64  # head dim
DG = HG * DK  # 512, per-core projection width
ROPE_THETA = 10000.0
P = 128  # partitions
NS = S // P  # 8 s-tiles
ND = D // P  # 8 d-chunks
F32 = mybir.dt.float32
BF16 = mybir.dt.bfloat16
EXP = mybir.ActivationFunctionType.Exp

_uid = [0]


def _split_excess_waits(nc, limit=1):
    """This container's walrus rejects >1 sync waits on the kernel-tail
    Drain; move excess waits onto same-engine NoOps inserted before it."""
    for f in nc.m.functions:
        for blk in f.blocks:
            insts = list(blk.instructions)
            out = []
            changed = False
            for inst in insts:
                si = inst.sync_info
                if si is not None and si.on_wait and len(si.on_wait) > limit:
                    waits = list(si.on_wait)
                    head, tail = waits[:-limit], waits[-limit:]
                    for i in range(0, len(head), limit):
                        _uid[0] += 1
                        nop = mybir.InstNoOp(
                            name=f"waitsplit-{_uid[0]}", ins=[], outs=[]
                        )
                        nop.engine = inst.engine
                        nop.sync_info = mybir.SyncInfo(
                            on_wait=head[i : i + limit], on_update=[]
                        )
                        out.append(nop)
                    si.on_wait = tail
                    changed = True
                out.append(inst)
            if changed:
                blk.instructions = out
    return nc


def build_nc():
    nc = bass.Bass("TRN2")
    xT = nc.dram_tensor("xT", [D, S], BF16, kind="ExternalInput")
    wqT = nc.dram_tensor("wqT", [D, DG], BF16, kind="ExternalInput")
    wkT = nc.dram_tensor("wkT", [D, DG], BF16, kind="ExternalInput")
    wvT = nc.dram_tensor("wvT", [D, DG], BF16, kind="ExternalInput")
    woT = nc.dram_tensor("woT", [DG, D], BF16, kind="ExternalInput")
    cosT = nc.dram_tensor("cosT", [P, S], BF16, kind="ExternalInput")
    sinTs = nc.dram_tensor("sinTs", [P, S], BF16, kind="ExternalInput")
    pswT = nc.dram_tensor("pswT", [P, P], BF16, kind="ExternalInput")
    yT = nc.dram_tensor("yT", [D, S], BF16, kind="ExternalOutput")
    # DRAM scratch for the softmax 1/sum reshape + partition-broadcast
    # (one 1024-wide slot per head-pair-half: [0:512]=odd head, [512:]=even)
    rsum = nc.dram_tensor("rsum", [HG, 2 * 512], F32)
    rbcd = nc.dram_tensor("rbcd", [HG, 2 * 512], F32)

    with tile.TileContext(nc) as tc:
        with (
            tc.tile_pool(name="const", bufs=1) as constp,
            tc.tile_pool(name="wts", bufs=1) as wp,
            tc.tile_pool(name="big", bufs=1) as bigp,
            tc.tile_pool(name="qsw", bufs=3) as qswp,
            tc.tile_pool(name="rr", bufs=2) as rrp,
            tc.tile_pool(name="ysb", bufs=3) as ysp,
        ):
            # ---- constants ----
            ztrilf = constp.tile([P, P], F32, tag="ztrilf")
            nc.vector.memset(ztrilf[:, :], 0.0)
            make_upper_triangular(nc, ztrilf[:, :], val=1.0, diag=True)
            ztril = constp.tile([P, P], BF16, tag="ztril")
            nc.vector.tensor_copy(ztril[:, :], ztrilf[:, :])

            # ---- resident weights/activations ----
            xs = wp.tile([P, ND, S], BF16, tag="xs", name="xs")
            wq_all = wp.tile([P, ND, DG], BF16, tag="wq", name="wq")
            wk_all = wp.tile([P, ND, DG], BF16, tag="wk", name="wk")
            wv_all = wp.tile([P, ND, DG], BF16, tag="wv", name="wv")
            wo_all = wp.tile([P, DG // P, D], BF16, tag="wo", name="wo")
            cs = wp.tile([P, S], BF16, tag="cs", name="cs")
            sn = wp.tile([P, S], BF16, tag="sn", name="sn")
            psw = wp.tile([P, P], BF16, tag="psw", name="psw")

            # xs per-chunk on the sync queue (first V matmuls unblock
            # early); weights on the ACT hwdge queue run concurrently.
            for c in range(ND):
                nc.sync.dma_start(out=xs[:, c, :], in_=xT[ts(c, P), :])
            nc.scalar.dma_start(
                out=wv_all[:, :, :],
                in_=wvT[:, :].rearrange("(c p) o -> p c o", p=P),
            )
            nc.scalar.dma_start(out=psw[:, :], in_=pswT[:, :])
            nc.scalar.dma_start(out=cs[:, :], in_=cosT[:, :])
            nc.scalar.dma_start(out=sn[:, :], in_=sinTs[:, :])
            nc.scalar.dma_start(
                out=wq_all[:, :, :],
                in_=wqT[:, :].rearrange("(c p) o -> p c o", p=P),
            )
            nc.sync.dma_start(
                out=wk_all[:, :, :],
                in_=wkT[:, :].rearrange("(c p) o -> p c o", p=P),
            )
            nc.scalar.dma_start(
                out=wo_all[:, :, :],
                in_=woT[:, :].rearrange("(c p) o -> p c o", p=P),
            )

            # persistent: q^T/k^T pair tiles [128 dims, S], v tiles, at tiles
            qt_sb = [bigp.tile([P, S], BF16, tag=f"qt{p}", name=f"qt{p}") for p in range(4)]
            kt_sb = [bigp.tile([P, S], BF16, tag=f"kt{p}", name=f"kt{p}") for p in range(4)]
            v_sb = [bigp.tile([P, HG, DK + 1], BF16, tag=f"v{j}", name=f"v{j}") for j in range(NS)]
            at_sb = [bigp.tile([P, S], BF16, tag=f"at{p}", name=f"at{p}") for p in range(4)]
            for j in range(NS):
                nc.vector.memset(v_sb[j][:, :, DK : DK + 1], 1.0)

            def normalize_pair(ha, apa, hb, apb, hx, eng, dmae=None):
                # batched softmax 1/sum for a head pair: copy both s-rows
                # into one tile (partitions 0 and 32), reshape via DRAM to
                # [128,8], one reciprocal, broadcast back, two multiplies.
                # Odd heads write at partitions 64:128 directly.
                sr2 = rrp.tile([P, 512], F32, tag="sr", name="sr2")
                if eng == "act":
                    nc.scalar.copy(out=sr2[0:1, :], in_=apa[DK : DK + 1, 0:512])
                    nc.scalar.copy(out=sr2[32:33, :], in_=apb[DK : DK + 1, 0:512])
                else:
                    nc.vector.tensor_copy(sr2[0:1, :], apa[DK : DK + 1, 0:512])
                    nc.vector.tensor_copy(sr2[32:33, :], apb[DK : DK + 1, 0:512])
                dq = nc.sync if dmae is None else nc.scalar
                slot = (ha // 2) + (4 if hx else 0)
                src = bass.AP(
                    tensor=sr2[:, :].tensor,
                    offset=sr2[:, :].offset,
                    ap=[[32 * 512, 2], [1, 512]],
                )
                dq.dma_start(
                    out=rsum[slot, :].rearrange("(o c) -> o c", o=2), in_=src
                )
                rs = rrp.tile([P, 8], F32, tag="rs", name="rs")
                dq.dma_start(
                    out=rs[:, :], in_=rsum[slot, :].rearrange("(p c) -> p c", c=8)
                )
                rc = rrp.tile([P, 8], F32, tag="rc", name="rc")
                nc.vector.reciprocal(out=rc[:, :], in_=rs[:, :])
                dq.dma_start(
                    out=rbcd[slot, :].rearrange("(p c) -> p c", c=8), in_=rc[:, :]
                )
                row = rbcd[slot, :]
                bc_src = bass.AP(
                    tensor=row.tensor, offset=row.offset, ap=[[0, DK], [1, 1024]]
                )
                rbc2 = rrp.tile([DK, 1024], F32, tag="rbc", name="rbc2")
                dq.dma_start(out=rbc2[:, :], in_=bc_src)
                for h, ap, col in ((ha, apa, 0), (hb, apb, 512)):
                    pair, poff = h // 2, 64 * (h % 2)
                    nc.vector.tensor_mul(
                        at_sb[pair][poff : poff + DK, hx : hx + 512],
                        ap[0:DK, 0:512],
                        rbc2[:, col : col + 512],
                    )

            with (
                tc.tile_pool(name="projv", bufs=3, space="PSUM") as pvp,
                tc.tile_pool(name="lg0", bufs=2, space="PSUM") as lg0p,
                tc.tile_pool(name="ap0", bufs=3, space="PSUM") as ap0p,
                tc.tile_pool(name="pt0", bufs=5) as pt0p,
            ):
                def emit_v(i):
                    vp = pvp.tile([P, DG], F32, tag="pv", name=f"v{i}")
                    for c in range(ND):
                        nc.tensor.matmul(
                            vp[:, :], lhsT=xs[:, c, ts(i, P)], rhs=wv_all[:, c, :],
                            start=(c == 0), stop=(c == ND - 1),
                        )
                    nc.vector.tensor_copy(
                        v_sb[i][:, :, 0:DK],
                        vp[:, :].rearrange("p (h c) -> p h c", h=HG),
                    )

                def emit_qkt(src, pair, hx):
                    # transposed projection of q or k pair tile, cols [hx, hx+512)
                    w_all = wq_all if src == "q" else wk_all
                    dst = qt_sb[pair] if src == "q" else kt_sb[pair]
                    pp = pvp.tile([P, DG], F32, tag="pv", name=f"{src}{pair}_{hx}")
                    for c in range(ND):
                        nc.tensor.matmul(
                            pp[:, :], lhsT=w_all[:, c, ts(pair, P)],
                            rhs=xs[:, c, hx : hx + 512],
                            start=(c == 0), stop=(c == ND - 1),
                        )
                    # rope: dst = pp*cos + blockswap(pp)*sin_signed; the
                    # block swap is a PE permutation matmul (psw)
                    qs = qswp.tile([P, 512], BF16, tag="qs", name="qs")
                    nc.scalar.copy(out=qs[:, :], in_=pp[:, :])
                    qw = pvp.tile([P, DG], F32, tag="pv", name="qw")
                    nc.tensor.matmul(
                        qw[:, :], lhsT=psw[:, :], rhs=qs[:, :],
                        start=True, stop=True,
                    )
                    t1 = qswp.tile([P, 512], BF16, tag="t1", name="t1")
                    nc.vector.tensor_mul(t1[:, :], qs[:, :], cs[:, hx : hx + 512])
                    t2 = qswp.tile([P, 512], BF16, tag="t2", name="t2")
                    nc.vector.tensor_mul(t2[:, :], qw[:, :], sn[:, hx : hx + 512])
                    nc.gpsimd.tensor_add(dst[:, hx : hx + 512], t1[:, :], t2[:, :])

                def ev0(h, ap, j, q0, n, pt):
                    nc.tensor.matmul(
                        ap[0 : DK + 1, q0:512],
                        lhsT=v_sb[j][:, h, :], rhs=pt[:, 0:n],
                        start=(j == 0), stop=(j == 3),
                        skip_group_check=True,
                    )

                def emit_h0_head(h, extra=None):
                    # half-0: q in [0,512), k-tiles 0..3
                    pair, poff = h // 2, 64 * (h % 2)
                    ap = ap0p.tile([P, 512], F32, tag="ap", name=f"ap0_{h}")
                    pend = []
                    for j in range(4):
                        q0 = 128 * j
                        n = 512 - q0
                        lg = lg0p.tile([P, 512], F32, tag="lg", name="lg0")
                        nc.tensor.matmul(
                            lg[:, 0:n],
                            lhsT=kt_sb[pair][poff : poff + DK, ts(j, P)],
                            rhs=qt_sb[pair][poff : poff + DK, q0:512],
                            start=True, stop=True,
                        )
                        pt = pt0p.tile([P, 512], BF16, tag="pt", name="pt0")
                        nc.scalar.activation(
                            out=pt[:, 0:n], in_=lg[:, 0:n], func=EXP, scale=0.125
                        )
                        nc.gpsimd.tensor_mul(pt[:, 0:P], pt[:, 0:P], ztril[:, :])
                        pend.append((j, q0, n, pt))
                        while len(pend) > 2:
                            ev0(h, ap, *pend.pop(0))
                    if extra is not None:
                        extra()  # PE filler while the tail exp/tril drain
                    for args in pend:
                        ev0(h, ap, *args)
                    return ap

                # ---- emission: phase A half-0, half-0 attn, phase A half-1
                for i in range(4):
                    emit_v(i)
                aps0 = {}
                for pair in range(4):
                    emit_qkt("q", pair, 0)
                    emit_qkt("k", pair, 0)
                    if pair == 1:
                        aps0[1] = emit_h0_head(1)
                    elif pair == 2:
                        aps0[0] = emit_h0_head(0)
                        normalize_pair(1, aps0.pop(1), 0, aps0.pop(0), 0, "act")
                    elif pair == 3:
                        aps0[3] = emit_h0_head(3)
                emit_v(4)
                aps0[2] = emit_h0_head(2)
                normalize_pair(3, aps0.pop(3), 2, aps0.pop(2), 0, "act")
                emit_v(5)
                aps0[5] = emit_h0_head(5, extra=lambda: (
                    emit_qkt("q", 0, 512), emit_qkt("k", 0, 512)))
                emit_v(6)
                aps0[4] = emit_h0_head(4, extra=lambda: (
                    emit_qkt("q", 1, 512), emit_qkt("k", 1, 512)))
                normalize_pair(5, aps0.pop(5), 4, aps0.pop(4), 0, "act")
                emit_v(7)
                aps0[7] = emit_h0_head(7, extra=lambda: (
                    emit_qkt("q", 2, 512), emit_qkt("k", 2, 512)))
                aps0[6] = emit_h0_head(6, extra=lambda: (
                    emit_qkt("q", 3, 512), emit_qkt("k", 3, 512)))
                normalize_pair(7, aps0.pop(7), 6, aps0.pop(6), 0, "act", "scalar")

            # ---- half-1 attention + output projection ----
            with (
                tc.tile_pool(name="ypt", bufs=2, space="PSUM") as yptp,
                tc.tile_pool(name="lg1", bufs=3, space="PSUM") as lg1p,
                tc.tile_pool(name="ap1", bufs=3, space="PSUM") as ap1p,
                tc.tile_pool(name="pt1", bufs=6) as pt1p,
            ):
                def h1_qk(h, j):
                    # one k-tile of half-1 QK + exp (+ tril for diag tiles)
                    pair, poff = h // 2, 64 * (h % 2)
                    lo = max(512, 128 * j)
                    n = 1024 - lo
                    lg = lg1p.tile([P, 512], F32, tag="lg", name="lg1")
                    nc.tensor.matmul(
                        lg[:, 0:n],
                        lhsT=kt_sb[pair][poff : poff + DK, ts(j, P)],
                        rhs=qt_sb[pair][poff : poff + DK, lo:1024],
                        start=True, stop=True,
                    )
                    pt = pt1p.tile([P, 512], BF16, tag="pt", name="pt1")
                    nc.scalar.activation(
                        out=pt[:, 0:n], in_=lg[:, 0:n], func=EXP, scale=0.125
                    )
                    if 128 * j >= 512:  # diagonal block leads this tile
                        nc.gpsimd.tensor_mul(pt[:, 0:P], pt[:, 0:P], ztril[:, :])
                    return (j, lo, n, pt)

                def ev1(h, ap, j, lo, n, pt):
                    nc.tensor.matmul(
                        ap[0 : DK + 1, lo - 512 : 512],
                        lhsT=v_sb[j][:, h, :], rhs=pt[:, 0:n],
                        start=(j == 0), stop=(j == NS - 1),
                        skip_group_check=True,
                    )

                def emit_h1_pair(ha, hb, extra=None, dmae=None):
                    # two heads of one pair interleaved per k-tile: doubles
                    # the independent work in flight
                    apa = ap1p.tile([P, 512], F32, tag="ap", name=f"ap1_{ha}")
                    apb = ap1p.tile([P, 512], F32, tag="ap", name=f"ap1_{hb}")
                    pend = []
                    for j in range(NS):
                        pend.append((ha, apa) + h1_qk(ha, j))
                        pend.append((hb, apb) + h1_qk(hb, j))
                        if j == 1 and extra is not None:
                            extra()  # PE filler while the exp pipeline fills
                        while len(pend) > 4:
                            a = pend.pop(0)
                            ev1(a[0], a[1], *a[2:])
                    for a in pend:
                        ev1(a[0], a[1], *a[2:])
                    normalize_pair(ha, apa, hb, apb, 512, "dve", dmae)

                def emit_outproj(o, hx, copy_eng):
                    ypt = yptp.tile([P, 512], F32, tag="y", name=f"y{o}_{hx}")
                    for c in range(DG // P):
                        nc.tensor.matmul(
                            ypt[:, :],
                            lhsT=wo_all[:, c, ts(o, P)],
                            rhs=at_sb[c][:, hx : hx + 512],
                            start=(c == 0), stop=(c == DG // P - 1),
                        )
                    ysb = ysp.tile([P, 512], BF16, tag="ysb", name="ysb")
                    if copy_eng == "act":
                        nc.scalar.copy(out=ysb[:, :], in_=ypt[:, :])
                    else:
                        nc.vector.tensor_copy(ysb[:, :], ypt[:, :])
                    nc.sync.dma_start(out=yT[ts(o, P), hx : hx + 512], in_=ysb[:, :])

                for pr in range(4):
                    emit_h1_pair(
                        2 * pr + 1, 2 * pr,
                        extra=lambda pr=pr: (
                            emit_outproj(2 * pr, 0, "dve"),
                            emit_outproj(2 * pr + 1, 0, "dve"),
                        ),
                        dmae="scalar" if pr == 3 else None,
                    )
                for o in range(ND):
                    emit_outproj(o, 512, "act" if o % 2 else "dve")

    _split_excess_waits(nc)
    return nc


_NC_CACHE = {}


def _get_nc():
    if "nc" not in _NC_CACHE:
        _NC_CACHE["nc"] = build_nc()
    return _NC_CACHE["nc"]


# rotate-half permutation within each head: evens then odds
_PERM = np.concatenate([np.arange(0, DK, 2), np.arange(1, DK, 2)])


def _bf16(a):
    import ml_dtypes

    return np.asarray(a, dtype=ml_dtypes.bfloat16)


def _host_prep(x, Wq, Wk, Wv, Wo, token_positions):
    """Build the 8 per-core input dicts."""
    inv_freq = 1.0 / (ROPE_THETA ** (np.arange(0, DK, 2, dtype=np.float32) / DK))
    in_maps = []
    for core in range(8):
        b, g = core // 2, core % 2
        heads = np.arange(HG * g, HG * (g + 1))
        rows_qk = (heads[:, None] * DK + _PERM[None, :]).reshape(-1)
        rows_v = (heads[:, None] * DK + np.arange(DK)[None, :]).reshape(-1)
        pos = token_positions[b].astype(np.float32)  # [S]
        ang = pos[None, :] * inv_freq[:, None]  # [32, S]
        cosT = np.tile(np.cos(ang), (4, 1)).astype(np.float32)  # [128, S]
        sin = np.sin(ang)
        sinTs = np.concatenate([-sin, sin, -sin, sin], axis=0).astype(np.float32)
        psw = np.zeros((P, P), dtype=np.float32)
        psw[np.arange(P) ^ 32, np.arange(P)] = 1.0
        in_maps.append(
            {
                "xT": _bf16(x[b].T),
                "wqT": _bf16(Wq[rows_qk, :].T),
                "wkT": _bf16(Wk[rows_qk, :].T),
                "wvT": _bf16(Wv[rows_v, :].T),
                "woT": _bf16(Wo[:, rows_v].T),
                "cosT": _bf16(cosT),
                "sinTs": _bf16(sinTs),
                "pswT": _bf16(psw),
            }
        )
    return in_maps


def kernel(x, Wq, Wk, Wv, Wo, token_positions, _trace=False):
    x = np.asarray(x, dtype=np.float32)
    Wq = np.asarray(Wq, dtype=np.float32)
    Wk = np.asarray(Wk, dtype=np.float32)
    Wv = np.asarray(Wv, dtype=np.float32)
    Wo = np.asarray(Wo, dtype=np.float32)
    token_positions = np.asarray(token_positions)

    nc = _get_nc()
    in_maps = _host_prep(x, Wq, Wk, Wv, Wo, token_positions)
    res = run_bass_kernel_spmd(nc, in_maps, core_ids=list(range(8)), trace=_trace)
    if _trace:
        kernel.last_exec_time_ns = res.exec_time_ns
        kernel.last_results = res

    y = np.empty((B, S, D), dtype=np.float32)
    for b in range(B):
        yT0 = np.asarray(res.results[2 * b]["yT"], dtype=np.float32)
        yT1 = np.asarray(res.results[2 * b + 1]["yT"], dtype=np.float32)
        y[b] = (yT0 + yT1).T
    return y
